# revision 1
# baseline (speedup 1.0000x reference)
"""BiRNN (tanh SimpleRNN, both directions) as a Bass/Tile kernel on 8 trn2 cores.

Problem: x [64, 512, 512] fp32; per direction W [512,512], U [512,512], b [512].
  fw:  h_t = tanh(x_t @ Wf + h_{t-1} @ Uf + bf),  ys_fw[t] = h_t
  bw:  same over time-reversed x, outputs kept in loop order.
  out[b, t, :] = concat(fw[t, b], bw[t, b])  -> [64, 512, 1024] fp32

Sharding: 8 cores = 2 directions x 4 batch groups of 16. Weights replicated
per direction; the time recurrence stays on-core (cannot be sharded).

Per-core device program (SPMD; per-core differences are data only -- bw cores
receive time-reversed x and the bw weights):
  1. xw^T precompute: psum[m] += Wt[k,m].T @ xT[k]  (fp16 operands, fp32 psum),
     stored fp16 in SBUF as 16 tiles hx[m][j]: [128 h, 16 b, 128 t].
  2. 512 sequential steps, state kept transposed (h^T: partitions = hidden):
     psum[m] += Ut[k,m].T @ h^T[k]   (16 LDW+MM pairs per step, LDW-bound)
     tmp = psum[m] + xw^T cols (DVE), h^T[m] = tanh(tmp + b) (ACT), written
     in place over the consumed xw column so the same buffer accumulates the
     outputs.
  3. Per 128-step block, DMA the finished [128, 16, 128] tiles to DRAM fp16.

Host: pre-transposes/casts inputs per core, gathers [4,4,128,16,128] fp16
outputs, reassembles the [64, 512, 1024] fp32 result.
"""

import numpy as np

B, T, F, H = 64, 512, 512, 512
NCORES = 8
NGROUP = 4            # batch groups
BL = B // NGROUP      # 16 batch rows per core
KC = F // 128         # 4 contraction chunks
MC = H // 128         # 4 output chunks

_PROGRAM_CACHE = {}


def _build_program(steps=T):
    import concourse.mybir as mybir
    import concourse.tile as tile
    from concourse import bacc

    f16 = mybir.dt.float16
    f32 = mybir.dt.float32
    Tanh = mybir.ActivationFunctionType.Tanh
    nblocks = steps // 128

    nc = bacc.Bacc("TRN2", target_bir_lowering=False, debug=False)

    xT = nc.dram_tensor("xT", [KC, 128, BL, steps], f16, kind="ExternalInput").ap()
    Wt = nc.dram_tensor("Wt", [KC, MC, 128, 128], f16, kind="ExternalInput").ap()
    Ut = nc.dram_tensor("Ut", [KC, MC, 128, 128], f16, kind="ExternalInput").ap()
    bT = nc.dram_tensor("bT", [MC, 128, 1], f32, kind="ExternalInput").ap()
    ys = nc.dram_tensor(
        "ys", [MC, nblocks, 128, BL, 128], f16, kind="ExternalOutput"
    ).ap()

    with tile.TileContext(nc) as tc:
        with (
            tc.tile_pool(name="weights", bufs=1) as wpool,
            tc.tile_pool(name="xbuf", bufs=1) as xpool,
            tc.tile_pool(name="hxbuf", bufs=1) as hxpool,
            tc.tile_pool(name="tmp", bufs=8) as tmppool,
        ):
            W_sb = [
                [
                    wpool.tile([128, 128], f16, tag=f"W{k}{m}", name=f"W_sb{k}{m}")
                    for m in range(MC)
                ]
                for k in range(KC)
            ]
            for k in range(KC):
                for m in range(MC):
                    nc.sync.dma_start(W_sb[k][m][:], Wt[k, m])
            x_sb = [
                xpool.tile([128, BL, steps], f16, tag=f"x{k}", name=f"x_sb{k}")
                for k in range(KC)
            ]
            for k in range(KC):
                nc.sync.dma_start(x_sb[k][:], xT[k])
            U_sb = [
                [
                    wpool.tile([128, 128], f16, tag=f"U{k}{m}", name=f"U_sb{k}{m}")
                    for m in range(MC)
                ]
                for k in range(KC)
            ]
            for k in range(KC):
                for m in range(MC):
                    nc.sync.dma_start(U_sb[k][m][:], Ut[k, m])
            b_sb = [
                wpool.tile([128, 1], f32, tag=f"b{m}", name=f"b_sb{m}")
                for m in range(MC)
            ]
            for m in range(MC):
                nc.sync.dma_start(b_sb[m][:], bT[m])

            # xw^T / h^T output buffer: hx[m][j] covers hidden chunk m,
            # time block j; layout [128 h, 16 b, 128 t].
            hx = [
                [
                    hxpool.tile([128, BL, 128], f16, tag=f"hx{m}_{j}", name=f"hx{m}_{j}")
                    for j in range(nblocks)
                ]
                for m in range(MC)
            ]

            # Phase 1: input projection xw^T = (x @ W)^T.
            with tc.tile_pool(name="pcpsum", bufs=4, space="PSUM") as pcpool:
                for j in range(nblocks):
                    for m in range(MC):
                        for n in range(BL // 4):
                            ps = pcpool.tile(
                                [128, 4, 128], f32, tag="pc", name=f"pc{j}_{m}_{n}"
                            )
                            for k in range(KC):
                                nc.tensor.matmul(
                                    ps[:],
                                    W_sb[k][m][:],
                                    x_sb[k][:, 4 * n : 4 * n + 4, 128 * j : 128 * j + 128],
                                    start=(k == 0),
                                    stop=(k == KC - 1),
                                )
                            nc.vector.tensor_copy(hx[m][j][:, 4 * n : 4 * n + 4, :], ps[:])

            # Phase 2: the recurrence.
            with tc.tile_pool(name="rpsum", bufs=2, space="PSUM") as rpool:
                for t in range(steps):
                    j, tl = divmod(t, 128)
                    if t == 0:
                        for m in range(MC):
                            nc.scalar.activation(
                                hx[m][0][:, :, 0],
                                hx[m][0][:, :, 0],
                                Tanh,
                                bias=b_sb[m][:],
                            )
                    else:
                        jp, tlp = divmod(t - 1, 128)
                        ps_t = [
                            rpool.tile([128, BL], f32, tag=f"ps{m}", name=f"ps{m}_{t}")
                            for m in range(MC)
                        ]
                        # k-outer so next step's k=0 matmuls only need the
                        # first tanh chunk of this step (pipelines PE vs ACT).
                        for k in range(KC):
                            hprev = hx[k][jp][:, :, tlp]
                            for m in range(MC):
                                nc.tensor.matmul(
                                    ps_t[m][:],
                                    U_sb[k][m][:],
                                    hprev,
                                    start=(k == 0),
                                    stop=(k == KC - 1),
                                )
                        for m in range(MC):
                            tmp = tmppool.tile(
                                [128, BL], f32, tag="tmp", name=f"tmp{m}_{t}"
                            )
                            nc.vector.tensor_add(tmp[:], ps_t[m][:], hx[m][j][:, :, tl])
                            nc.scalar.activation(
                                hx[m][j][:, :, tl], tmp[:], Tanh, bias=b_sb[m][:]
                            )
                    if tl == 127:
                        for m in range(MC):
                            nc.sync.dma_start(ys[m, j], hx[m][j][:])

    nc.compile()
    return nc


def get_program(steps=T):
    if steps not in _PROGRAM_CACHE:
        _PROGRAM_CACHE[steps] = _build_program(steps)
    return _PROGRAM_CACHE[steps]


def make_in_maps(x, Wf, Uf, bf, Wb, Ub, bb, steps=T):
    """Per-core input dicts. Core c: direction c//4 (0 fw, 1 bw), batch group c%4."""
    x = np.asarray(x, dtype=np.float32)
    in_maps = []
    for c in range(NCORES):
        d, g = divmod(c, NGROUP)
        xs = x[g * BL : (g + 1) * BL, :steps]
        if d == 1:
            xs = xs[:, ::-1]
        # xT[k, p, b, t] = xs[b, t, 128k + p]
        xTc = xs.transpose(2, 0, 1).astype(np.float16).reshape(KC, 128, BL, steps)
        W, U, bvec = (Wf, Uf, bf) if d == 0 else (Wb, Ub, bb)
        Wtc = np.ascontiguousarray(
            np.asarray(W, np.float32).reshape(KC, 128, MC, 128).transpose(0, 2, 1, 3)
        ).astype(np.float16)
        Utc = np.ascontiguousarray(
            np.asarray(U, np.float32).reshape(KC, 128, MC, 128).transpose(0, 2, 1, 3)
        ).astype(np.float16)
        bTc = np.asarray(bvec, np.float32).reshape(MC, 128, 1)
        in_maps.append({"xT": xTc, "Wt": Wtc, "Ut": Utc, "bT": bTc})
    return in_maps


def assemble_output(per_core_ys, steps=T):
    out = np.empty((B, steps, 2 * H), dtype=np.float32)
    for c in range(NCORES):
        d, g = divmod(c, NGROUP)
        ysc = np.asarray(per_core_ys[c])  # [MC, nblocks, 128, BL, 128] fp16
        y = ysc.transpose(3, 1, 4, 0, 2).reshape(BL, steps, H).astype(np.float32)
        out[g * BL : (g + 1) * BL, :, d * H : (d + 1) * H] = y
    return out


def kernel(**inputs):
    nc = get_program(T)
    in_maps = make_in_maps(
        inputs["x"], inputs["Wf"], inputs["Uf"], inputs["bf"],
        inputs["Wb"], inputs["Ub"], inputs["bb"],
    )
    from concourse.bass_utils import run_bass_kernel_spmd

    res = run_bass_kernel_spmd(nc, in_maps, list(range(NCORES)))
    return assemble_output([res.results[c]["ys"] for c in range(NCORES)])


# revision 2
# speedup vs baseline: 1.1966x; 1.1966x over previous
"""BiRNN (tanh SimpleRNN, both directions) as a Bass/Tile kernel on 8 trn2 cores.

Problem: x [64, 512, 512] fp32; per direction W [512,512], U [512,512], b [512].
  fw:  h_t = tanh(x_t @ Wf + h_{t-1} @ Uf + bf),  ys_fw[t] = h_t
  bw:  same over time-reversed x, outputs kept in loop order.
  out[b, t, :] = concat(fw[t, b], bw[t, b])  -> [64, 512, 1024] fp32

Sharding: 8 cores = 2 directions x 4 batch groups of 16. Weights replicated
per direction; the time recurrence stays on-core (cannot be sharded).

Per-core device program (SPMD; per-core differences are data only -- bw cores
receive time-reversed x and the bw weights):
  1. xw^T precompute: psum += Wt[k,m].T @ xT[k] (fp16 operands, fp32 psum),
     drained by DVE tensor_scalar_add(+bias) into fp16 SBUF tiles
     hx[j]: [128 h, 4 m, 16 b, 128 t].  Block 0 runs as a prologue; blocks
     1-3 are streamed one matmul per step into the recurrence's PE idle
     windows.
  2. 512 sequential steps, state kept transposed (h^T: partitions = hidden):
     psum[128, 4, 16]  = I128.T @ xw cols        (accumulation start)
     psum[:, m, :]    += Ut[k,m].T @ h^T[k]      (16 LDW+MM pairs)
     h^T cols          = tanh(psum)              (ONE activation, psum->SBUF,
                                                  overwrites the consumed xw
                                                  column in place)
  3. Per 128-step block, DMA the finished [128, 4, 16, 128] tile to DRAM fp16.

Host: pre-transposes/casts inputs per core, gathers [4,128,4,16,128] fp16
outputs, reassembles the [64, 512, 1024] fp32 result.
"""

import numpy as np

B, T, F, H = 64, 512, 512, 512
NCORES = 8
NGROUP = 4            # batch groups
BL = B // NGROUP      # 16 batch rows per core
KC = F // 128         # 4 contraction chunks
MC = H // 128         # 4 output chunks

_PROGRAM_CACHE = {}


def _build_program(steps=T):
    import concourse.mybir as mybir
    import concourse.tile as tile
    from concourse import bacc

    f16 = mybir.dt.float16
    f32 = mybir.dt.float32
    Tanh = mybir.ActivationFunctionType.Tanh
    nblocks = steps // 128

    nc = bacc.Bacc("TRN2", target_bir_lowering=False, debug=False)

    xT = nc.dram_tensor("xT", [KC, 128, BL, steps], f16, kind="ExternalInput").ap()
    Wt = nc.dram_tensor("Wt", [KC, MC, 128, 128], f16, kind="ExternalInput").ap()
    Ut = nc.dram_tensor("Ut", [KC, MC, 128, 128], f16, kind="ExternalInput").ap()
    bT = nc.dram_tensor("bT", [MC, 128, 1], f32, kind="ExternalInput").ap()
    eye = nc.dram_tensor("eye", [128, 128], f16, kind="ExternalInput").ap()
    ys = nc.dram_tensor(
        "ys", [nblocks, 128, MC, BL, 128], f16, kind="ExternalOutput"
    ).ap()

    with tile.TileContext(nc) as tc:
        with (
            tc.tile_pool(name="weights", bufs=1) as wpool,
            tc.tile_pool(name="xbuf", bufs=1) as xpool,
            tc.tile_pool(name="hxbuf", bufs=1) as hxpool,
            tc.tile_pool(name="pcpsum", bufs=2, space="PSUM") as pcpool,
            tc.tile_pool(name="rpsum", bufs=4, space="PSUM") as rpool,
        ):
            W_sb = [
                [
                    wpool.tile([128, 128], f16, tag=f"W{k}{m}", name=f"W_sb{k}{m}")
                    for m in range(MC)
                ]
                for k in range(KC)
            ]
            for k in range(KC):
                for m in range(MC):
                    nc.sync.dma_start(W_sb[k][m][:], Wt[k, m])
            x_sb = [
                xpool.tile([128, BL, steps], f16, tag=f"x{k}", name=f"x_sb{k}")
                for k in range(KC)
            ]
            for k in range(KC):
                nc.sync.dma_start(x_sb[k][:], xT[k])
            U_sb = [
                [
                    wpool.tile([128, 128], f16, tag=f"U{k}{m}", name=f"U_sb{k}{m}")
                    for m in range(MC)
                ]
                for k in range(KC)
            ]
            for k in range(KC):
                for m in range(MC):
                    nc.sync.dma_start(U_sb[k][m][:], Ut[k, m])
            b_sb = [
                wpool.tile([128, 1], f32, tag=f"b{m}", name=f"b_sb{m}")
                for m in range(MC)
            ]
            for m in range(MC):
                nc.sync.dma_start(b_sb[m][:], bT[m])
            eye_sb = wpool.tile([128, 128], f16, tag="eye", name="eye_sb")
            nc.sync.dma_start(eye_sb[:], eye[:])

            # xw^T / h^T buffer per time block: [128 h, 4 m, 16 b, 128 t]
            hx = [
                hxpool.tile([128, MC, BL, 128], f16, tag=f"hx{j}", name=f"hx{j}")
                for j in range(nblocks)
            ]

            def pc_unit_mm(j, m, n, k, ps):
                nc.tensor.matmul(
                    ps[:],
                    W_sb[k][m][:],
                    x_sb[k][:, 4 * n : 4 * n + 4, 128 * j : 128 * j + 128],
                    start=(k == 0),
                    stop=(k == KC - 1),
                )

            def pc_unit_drain(j, m, n, ps):
                # += bias while downcasting to fp16
                nc.vector.tensor_scalar_add(
                    hx[j][:, m, 4 * n : 4 * n + 4, :], ps[:], b_sb[m][:]
                )

            def pc_block_units(j):
                for m in range(MC):
                    for n in range(BL // 4):
                        yield (j, m, n)

            # Prologue: precompute block 0 fully.
            for (j, m, n) in pc_block_units(0):
                ps = pcpool.tile([128, 4, 128], f32, tag="pc", name=f"pc0_{m}_{n}")
                for k in range(KC):
                    pc_unit_mm(j, m, n, k, ps)
                pc_unit_drain(j, m, n, ps)

            # Streamed precompute state for blocks 1..nblocks-1
            pc_stream = None
            pc_ps = None

            def pc_step(jnext, s):
                # one pc matmul per early step; drain after each 4th
                nonlocal pc_stream, pc_ps
                if s == 0:
                    pc_stream = list(pc_block_units(jnext))
                if s < 64:
                    u, k = divmod(s, 4)
                    j, m, n = pc_stream[u]
                    if k == 0:
                        pc_ps = pcpool.tile(
                            [128, 4, 128], f32, tag="pc", name=f"pc{j}_{m}_{n}"
                        )
                    pc_unit_mm(j, m, n, k, pc_ps)
                    if k == KC - 1:
                        pc_unit_drain(j, m, n, pc_ps)

            # Recurrence.
            for t in range(steps):
                j, tl = divmod(t, 128)
                if t == 0:
                    nc.scalar.activation(
                        hx[0][:, :, :, 0], hx[0][:, :, :, 0], Tanh
                    )
                else:
                    jp, tlp = divmod(t - 1, 128)
                    ps_t = rpool.tile([128, MC, BL], f32, tag="ps", name=f"ps_{t}")
                    # xw injection: psum = I.T @ xw cols (whole tile, start)
                    nc.tensor.matmul(
                        ps_t[:],
                        eye_sb[:],
                        hx[j][:, :, :, tl],
                        start=True,
                        stop=False,
                        skip_group_check=True,
                    )
                    for k in range(KC):
                        hprev = hx[jp][:, k, :, tlp]
                        for m in range(MC):
                            nc.tensor.matmul(
                                ps_t[:, m, :],
                                U_sb[k][m][:],
                                hprev,
                                start=False,
                                stop=(k == KC - 1),
                                skip_group_check=True,
                            )
                    nc.scalar.activation(hx[j][:, :, :, tl], ps_t[:], Tanh)
                if j + 1 < nblocks:
                    pc_step(j + 1, tl)
                if tl == 127:
                    nc.sync.dma_start(ys[j], hx[j][:])

    nc.compile()
    return nc


def get_program(steps=T):
    if steps not in _PROGRAM_CACHE:
        _PROGRAM_CACHE[steps] = _build_program(steps)
    return _PROGRAM_CACHE[steps]


def make_in_maps(x, Wf, Uf, bf, Wb, Ub, bb, steps=T):
    """Per-core input dicts. Core c: direction c//4 (0 fw, 1 bw), batch group c%4."""
    x = np.asarray(x, dtype=np.float32)
    eye = np.eye(128, dtype=np.float16)
    in_maps = []
    for c in range(NCORES):
        d, g = divmod(c, NGROUP)
        xs = x[g * BL : (g + 1) * BL, :steps]
        if d == 1:
            xs = xs[:, ::-1]
        # xT[k, p, b, t] = xs[b, t, 128k + p]
        xTc = xs.transpose(2, 0, 1).astype(np.float16).reshape(KC, 128, BL, steps)
        W, U, bvec = (Wf, Uf, bf) if d == 0 else (Wb, Ub, bb)
        Wtc = np.ascontiguousarray(
            np.asarray(W, np.float32).reshape(KC, 128, MC, 128).transpose(0, 2, 1, 3)
        ).astype(np.float16)
        Utc = np.ascontiguousarray(
            np.asarray(U, np.float32).reshape(KC, 128, MC, 128).transpose(0, 2, 1, 3)
        ).astype(np.float16)
        bTc = np.asarray(bvec, np.float32).reshape(MC, 128, 1)
        in_maps.append({"xT": xTc, "Wt": Wtc, "Ut": Utc, "bT": bTc, "eye": eye})
    return in_maps


def assemble_output(per_core_ys, steps=T):
    out = np.empty((B, steps, 2 * H), dtype=np.float32)
    for c in range(NCORES):
        d, g = divmod(c, NGROUP)
        ysc = np.asarray(per_core_ys[c])  # [nblocks, 128, MC, BL, 128] fp16
        # out[b, 128j+tl, 128m+p] = ys[j, p, m, b, tl]
        y = ysc.transpose(3, 0, 4, 2, 1).reshape(BL, steps, H).astype(np.float32)
        out[g * BL : (g + 1) * BL, :, d * H : (d + 1) * H] = y
    return out


def kernel(**inputs):
    nc = get_program(T)
    in_maps = make_in_maps(
        inputs["x"], inputs["Wf"], inputs["Uf"], inputs["bf"],
        inputs["Wb"], inputs["Ub"], inputs["bb"],
    )
    from concourse.bass_utils import run_bass_kernel_spmd

    res = run_bass_kernel_spmd(nc, in_maps, list(range(NCORES)))
    return assemble_output([res.results[c]["ys"] for c in range(NCORES)])


# revision 3
# speedup vs baseline: 1.4410x; 1.2043x over previous
"""BiRNN (tanh SimpleRNN, both directions) as a Bass/Tile kernel on 8 trn2 cores.

Problem: x [64, 512, 512] fp32; per direction W [512,512], U [512,512], b [512].
  fw:  h_t = tanh(x_t @ Wf + h_{t-1} @ Uf + bf),  ys_fw[t] = h_t
  bw:  same over time-reversed x, outputs kept in loop order.
  out[b, t, :] = concat(fw[t, b], bw[t, b])  -> [64, 512, 1024] fp32

Sharding: 8 cores = 2 directions x 4 batch groups of 16. Weights replicated
per direction; the time recurrence stays on-core (cannot be sharded).

Per-core device program (SPMD; per-core differences are data only -- bw cores
receive time-reversed x and the bw weights):
  1. xw^T precompute: psum += Wt[k,m].T @ xTb[k] (fp16 operands, fp32 psum),
     drained by DVE tensor_scalar_add(+bias) into fp16 SBUF tiles
     xw[j]: [128 h, 4 m, 16 b, 128 t].  Block 0 runs as a prologue; later
     blocks stream one matmul per step into the recurrence's PE idle windows
     (x itself is double-buffered per block from DRAM).
  2. 512 sequential steps, state kept transposed (h^T: partitions = hidden):
     psum[128, 4, 16]  = I128.T @ xw cols         (accumulation start)
     psum[:, m, :]    += Ut[k,m].T @ ht_{t-1}[:, k, :]   (16 LDW+MM pairs)
     ht_t              = tanh(psum)               (ONE activation, psum ->
                                                   small contiguous SBUF tile)
     out[j] cols       = ht_t                     (DVE copy, off critical path)
  3. Per 128-step block, DMA the finished [128, 4, 16, 128] tile to DRAM fp16.

Host: pre-transposes/casts inputs per core, gathers [4,128,4,16,128] fp16
outputs, reassembles the [64, 512, 1024] fp32 result.
"""

import numpy as np

B, T, F, H = 64, 512, 512, 512
NCORES = 8
NGROUP = 4            # batch groups
BL = B // NGROUP      # 16 batch rows per core
KC = F // 128         # 4 contraction chunks
MC = H // 128         # 4 output chunks

_PROGRAM_CACHE = {}


def _build_program(steps=T):
    import concourse.mybir as mybir
    import concourse.tile as tile
    from concourse import bacc

    f16 = mybir.dt.float16
    f32 = mybir.dt.float32
    Tanh = mybir.ActivationFunctionType.Tanh
    nblocks = steps // 128

    nc = bacc.Bacc("TRN2", target_bir_lowering=False, debug=False)

    xTb = nc.dram_tensor(
        "xTb", [KC, nblocks, 128, BL, 128], f16, kind="ExternalInput"
    ).ap()
    Wt = nc.dram_tensor("Wt", [KC, MC, 128, 128], f16, kind="ExternalInput").ap()
    Ut = nc.dram_tensor("Ut", [KC, MC, 128, 128], f16, kind="ExternalInput").ap()
    bT = nc.dram_tensor("bT", [MC, 128, 1], f32, kind="ExternalInput").ap()
    eye = nc.dram_tensor("eye", [128, 128], f16, kind="ExternalInput").ap()
    ys = nc.dram_tensor(
        "ys", [nblocks, 128, MC, BL, 128], f16, kind="ExternalOutput"
    ).ap()

    with tile.TileContext(nc) as tc:
        with (
            tc.tile_pool(name="weights", bufs=1) as wpool,
            tc.tile_pool(name="xstage", bufs=2) as xpool,
            tc.tile_pool(name="xwbuf", bufs=1) as xwpool,
            tc.tile_pool(name="outbuf", bufs=1) as outpool,
            tc.tile_pool(name="htbuf", bufs=4) as htpool,
            tc.tile_pool(name="pcpsum", bufs=2, space="PSUM") as pcpool,
            tc.tile_pool(name="rpsum", bufs=4, space="PSUM") as rpool,
        ):
            W_sb = [
                [
                    wpool.tile([128, 128], f16, tag=f"W{k}{m}", name=f"W_sb{k}{m}")
                    for m in range(MC)
                ]
                for k in range(KC)
            ]
            for k in range(KC):
                for m in range(MC):
                    nc.sync.dma_start(W_sb[k][m][:], Wt[k, m])
            U_sb = [
                [
                    wpool.tile([128, 128], f16, tag=f"U{k}{m}", name=f"U_sb{k}{m}")
                    for m in range(MC)
                ]
                for k in range(KC)
            ]
            for k in range(KC):
                for m in range(MC):
                    nc.sync.dma_start(U_sb[k][m][:], Ut[k, m])
            b_sb = [
                wpool.tile([128, 1], f32, tag=f"b{m}", name=f"b_sb{m}")
                for m in range(MC)
            ]
            for m in range(MC):
                nc.sync.dma_start(b_sb[m][:], bT[m])
            eye_sb = wpool.tile([128, 128], f16, tag="eye", name="eye_sb")
            nc.sync.dma_start(eye_sb[:], eye[:])

            # xw^T buffer per time block (pc-written, I-MM read)
            xw = [
                xwpool.tile([128, MC, BL, 128], f16, tag=f"xw{j}", name=f"xw{j}")
                for j in range(nblocks)
            ]
            # output buffer per time block (DVE-written, DMA-read)
            outb = [
                outpool.tile([128, MC, BL, 128], f16, tag=f"out{j}", name=f"outb{j}")
                for j in range(nblocks)
            ]

            def x_dma(j):
                tiles = []
                for k in range(KC):
                    xs = xpool.tile(
                        [128, BL, 128], f16, tag=f"xs{k}", name=f"xs{k}_{j}"
                    )
                    nc.sync.dma_start(xs[:], xTb[k, j])
                    tiles.append(xs)
                return tiles

            def pc_unit_mm(xs_tiles, m, n, k, ps):
                nc.tensor.matmul(
                    ps[:],
                    W_sb[k][m][:],
                    xs_tiles[k][:, 4 * n : 4 * n + 4, :],
                    start=(k == 0),
                    stop=(k == KC - 1),
                )

            def pc_unit_drain(j, m, n, ps):
                # += bias while downcasting to fp16
                nc.vector.tensor_scalar_add(
                    xw[j][:, m, 4 * n : 4 * n + 4, :], ps[:], b_sb[m][:]
                )

            # Prologue: stage x block 0 and precompute it fully.
            xs_cur = x_dma(0)
            for m in range(MC):
                for n in range(BL // 4):
                    ps = pcpool.tile([128, 4, 128], f32, tag="pc", name=f"pc0_{m}_{n}")
                    for k in range(KC):
                        pc_unit_mm(xs_cur, m, n, k, ps)
                    pc_unit_drain(0, m, n, ps)

            # Streamed precompute state for blocks 1..nblocks-1
            pc_state = {}

            def pc_step(jnext, s):
                # stage DMAs at s==0, then one pc matmul per early step
                if s == 0:
                    pc_state["xs"] = x_dma(jnext)
                    pc_state["units"] = [
                        (m, n) for m in range(MC) for n in range(BL // 4)
                    ]
                if 8 <= s < 8 + 64:
                    u, k = divmod(s - 8, 4)
                    m, n = pc_state["units"][u]
                    if k == 0:
                        pc_state["ps"] = pcpool.tile(
                            [128, 4, 128], f32, tag="pc", name=f"pc{jnext}_{m}_{n}"
                        )
                    pc_unit_mm(pc_state["xs"], m, n, k, pc_state["ps"])
                    if k == KC - 1:
                        pc_unit_drain(jnext, m, n, pc_state["ps"])

            # Recurrence.
            ht_prev = None
            for t in range(steps):
                j, tl = divmod(t, 128)
                ht = htpool.tile([128, MC, BL], f16, tag="ht", name=f"ht{t}")
                if t == 0:
                    nc.scalar.activation(ht[:], xw[0][:, :, :, 0], Tanh)
                else:
                    jp, tlp = divmod(t - 1, 128)
                    ps_t = rpool.tile([128, MC, BL], f32, tag="ps", name=f"ps_{t}")
                    # xw injection: psum = I.T @ xw cols (whole tile, start)
                    nc.tensor.matmul(
                        ps_t[:],
                        eye_sb[:],
                        xw[j][:, :, :, tl],
                        start=True,
                        stop=False,
                        skip_group_check=True,
                    )
                    for k in range(KC):
                        hprev = ht_prev[:, k, :]
                        for m in range(MC):
                            nc.tensor.matmul(
                                ps_t[:, m, :],
                                U_sb[k][m][:],
                                hprev,
                                start=False,
                                stop=(k == KC - 1),
                                skip_group_check=True,
                            )
                    nc.scalar.activation(ht[:], ps_t[:], Tanh)
                nc.vector.tensor_copy(outb[j][:, :, :, tl], ht[:])
                ht_prev = ht
                if j + 1 < nblocks:
                    pc_step(j + 1, tl)
                if tl == 127:
                    nc.sync.dma_start(ys[j], outb[j][:])

    nc.compile()
    return nc


def get_program(steps=T):
    if steps not in _PROGRAM_CACHE:
        _PROGRAM_CACHE[steps] = _build_program(steps)
    return _PROGRAM_CACHE[steps]


def make_in_maps(x, Wf, Uf, bf, Wb, Ub, bb, steps=T):
    """Per-core input dicts. Core c: direction c//4 (0 fw, 1 bw), batch group c%4."""
    x = np.asarray(x, dtype=np.float32)
    eye = np.eye(128, dtype=np.float16)
    nblocks = steps // 128
    in_maps = []
    for c in range(NCORES):
        d, g = divmod(c, NGROUP)
        xs = x[g * BL : (g + 1) * BL, :steps]
        if d == 1:
            xs = xs[:, ::-1]
        # xTb[k, j, p, b, tl] = xs[b, 128j + tl, 128k + p]
        xTc = xs.transpose(2, 0, 1).astype(np.float16).reshape(KC, 128, BL, steps)
        xTbc = np.ascontiguousarray(
            xTc.reshape(KC, 128, BL, nblocks, 128).transpose(0, 3, 1, 2, 4)
        )
        W, U, bvec = (Wf, Uf, bf) if d == 0 else (Wb, Ub, bb)
        Wtc = np.ascontiguousarray(
            np.asarray(W, np.float32).reshape(KC, 128, MC, 128).transpose(0, 2, 1, 3)
        ).astype(np.float16)
        Utc = np.ascontiguousarray(
            np.asarray(U, np.float32).reshape(KC, 128, MC, 128).transpose(0, 2, 1, 3)
        ).astype(np.float16)
        bTc = np.asarray(bvec, np.float32).reshape(MC, 128, 1)
        in_maps.append({"xTb": xTbc, "Wt": Wtc, "Ut": Utc, "bT": bTc, "eye": eye})
    return in_maps


def assemble_output(per_core_ys, steps=T):
    out = np.empty((B, steps, 2 * H), dtype=np.float32)
    for c in range(NCORES):
        d, g = divmod(c, NGROUP)
        ysc = np.asarray(per_core_ys[c])  # [nblocks, 128, MC, BL, 128] fp16
        # out[b, 128j+tl, 128m+p] = ys[j, p, m, b, tl]
        y = ysc.transpose(3, 0, 4, 2, 1).reshape(BL, steps, H).astype(np.float32)
        out[g * BL : (g + 1) * BL, :, d * H : (d + 1) * H] = y
    return out


def kernel(**inputs):
    nc = get_program(T)
    in_maps = make_in_maps(
        inputs["x"], inputs["Wf"], inputs["Uf"], inputs["bf"],
        inputs["Wb"], inputs["Ub"], inputs["bb"],
    )
    from concourse.bass_utils import run_bass_kernel_spmd

    res = run_bass_kernel_spmd(nc, in_maps, list(range(NCORES)))
    return assemble_output([res.results[c]["ys"] for c in range(NCORES)])
